# revision 7
# baseline (speedup 1.0000x reference)
"""Trainium2 Bass kernel for a 2-layer TransformerConv GNN + attention pooling.

Strategy: 64 equal graphs of 128 nodes; edges are within-graph. Shard 8
graphs per NeuronCore (batch sharding). Per graph, the scatter-softmax
attention over edges is computed DENSELY as masked attention with an edge
multiplicity matrix A[dst, src] (edge counts), so everything is PE matmuls:

  S = (Q/sqrt(C)) K^T  (per head)           -> PE
  P = A * exp(S - rowmax(S)); Z = rowsum(P) -> ACT/DVE
  agg = (P/Z) @ V                           -> PE (via P^T transpose)

Activations are kept TRANSPOSED [D, nodes] so natural-layout weights are
the stationary matmul operand. float32r matmuls give full fp32 precision
at 1 cycle/row for free-dim >= 256; attention internals use bf16.
"""

import sys
import numpy as np

if "/opt/trn_rl_repo" not in sys.path:
    sys.path.insert(0, "/opt/trn_rl_repo")

import ml_dtypes  # noqa: F401
import concourse.bacc as bacc
import concourse.bass as bass
import concourse.mybir as mybir
import concourse.tile as tile
from concourse.bass_utils import run_bass_kernel_spmd

F32 = mybir.dt.float32
F32R = mybir.dt.float32r
BF16 = mybir.dt.bfloat16
AF = mybir.ActivationFunctionType
AX = mybir.AxisListType
ALU = mybir.AluOpType

# problem constants (hardcoded per contract)
B, L, D, H, E = 64, 128, 768, 2, 131072
N = B * L                 # 8192 nodes
C = D // H                # 384 per-head channels
N_CORES = 8
G = B // N_CORES          # 8 graphs per core
NPC = G * L               # 1024 nodes per core
KT = D // 128             # 6 k-tiles of 128
CT = C // 128             # 3 c-tiles per head
NCH = NPC // 512          # 2 node chunks of 512
SCALE = 1.0 / float(np.sqrt(C))

_CACHE = {}


def _bcast_cols(t, kt, col0, ngraph, rep):
    """AP over tile [128, KT, NPC] reading column col0 + g*L for each of
    `ngraph` graphs, each repeated `rep` times (step-0 inner dim)."""
    full = t[:]
    pstep = full.ap[0][0]
    off = full.offset + kt * NPC + col0
    return bass.AP(full.tensor, off, [[pstep, 128], [L, ngraph], [0, rep]])


def _build_program():
    nc = bacc.Bacc("TRN2", target_bir_lowering=False)

    # ---- DRAM I/O ----
    xT_d = nc.dram_tensor("xT", [D, NPC], F32R, kind="ExternalInput")
    a_d = nc.dram_tensor("acnt", [G, L, L], F32, kind="ExternalInput")
    wd = {}
    for l in ("1", "2"):
        for w in ("wq", "wk", "wv", "ws"):
            wd[w + l] = nc.dram_tensor(w + l, [D, D], F32R, kind="ExternalInput")
        for b in ("bq", "bk", "bs"):
            wd[b + l] = nc.dram_tensor(b + l, [D], F32, kind="ExternalInput")
        wd["bv" + l] = nc.dram_tensor("bv" + l, [D], F32R, kind="ExternalInput")
    atti_w = nc.dram_tensor("atti_w", [2 * D, D], F32R, kind="ExternalInput")
    atti_b = nc.dram_tensor("atti_b", [D], F32, kind="ExternalInput")
    atts_w = nc.dram_tensor("atts_w", [D, 1], F32, kind="ExternalInput")
    fc1_w = nc.dram_tensor("fc1_w", [D, D], F32R, kind="ExternalInput")
    fc1_b = nc.dram_tensor("fc1_b", [D], F32, kind="ExternalInput")
    fc2_w = nc.dram_tensor("fc2_w", [D, 3], F32R, kind="ExternalInput")
    fc2_b = nc.dram_tensor("fc2_b", [3], F32, kind="ExternalInput")
    eye_f = nc.dram_tensor("eye_f", [128, 128], F32R, kind="ExternalInput")
    ones_d = nc.dram_tensor("ones_d", [128, 128], F32R, kind="ExternalInput")
    eye_b = nc.dram_tensor("eye_b", [128, 128], BF16, kind="ExternalInput")
    out_d = nc.dram_tensor("out", [G, 3], F32, kind="ExternalOutput")

    with tile.TileContext(nc) as tc, (
        tc.tile_pool(name="wpool", bufs=3)) as wp, (
        tc.tile_pool(name="act", bufs=3)) as actp, (
        tc.tile_pool(name="qk", bufs=1)) as qkp, (
        tc.tile_pool(name="small", bufs=1)) as sp, (
        tc.tile_pool(name="attn", bufs=4)) as ap_, (
        tc.tile_pool(name="psum", bufs=8, space=bass.MemorySpace.PSUM)) as pp:

        # ---- constants / small tiles ----
        eyeF = sp.tile([128, 128], F32R, tag="eyeF")
        nc.sync.dma_start(eyeF[:], eye_f[:, :])
        eyeB = sp.tile([128, 128], BF16, tag="eyeB")
        nc.sync.dma_start(eyeB[:], eye_b[:, :])
        ones_row = sp.tile([1, 128], F32R, tag="ones_row")
        nc.sync.dma_start(ones_row[:], ones_d[0:1, :])
        ones_row_f = sp.tile([1, 128], F32, tag="ones_row_f")
        nc.sync.dma_start(ones_row_f[:], ones_d[0:1, :].bitcast(F32))
        ones_col_f = sp.tile([128, 1], F32, tag="ones_col_f")
        nc.sync.dma_start(ones_col_f[:], ones_d[:, 0:1].bitcast(F32))

        def load_bias_cols(name, dram):
            t = sp.tile([128, KT], F32, tag="b_" + name)
            nc.sync.dma_start(t[:], dram[:].rearrange("(t p) -> p t", p=128))
            return t

        bias_c = {}
        for l in ("1", "2"):
            for b in ("bq", "bk", "bs"):
                bias_c[b + l] = load_bias_cols(b + l, wd[b + l])
            t = sp.tile([1, D], F32R, tag="br_bv" + l)
            nc.sync.dma_start(t[:], wd["bv" + l][:].rearrange("(a d) -> a d", a=1))
            bias_c["bv" + l] = t
            # pre-scaled bq for folding 1/sqrt(C) into q
            t = sp.tile([128, KT], F32, tag="bqs" + l)
            nc.vector.tensor_scalar_mul(t[:], bias_c["bq" + l][:], SCALE)
            bias_c["bqs" + l] = t
        attib_c = load_bias_cols("attib", atti_b)
        attsw_c = sp.tile([128, KT], F32, tag="attsw")
        nc.sync.dma_start(attsw_c[:],
                          atts_w[:, :].rearrange("(t p) o -> p (t o)", p=128))
        attsw_b = sp.tile([128, KT], BF16, tag="attswb")
        nc.vector.tensor_copy(attsw_b[:], attsw_c[:])
        fc1b_c = load_bias_cols("fc1b", fc1_b)
        fc2w_c = sp.tile([128, KT, 3], F32R, tag="fc2w")
        nc.sync.dma_start(fc2w_c[:],
                          fc2_w[:, :].rearrange("(t p) o -> p t o", p=128))
        fc2b_c = sp.tile([3, 1], F32, tag="fc2b")
        nc.sync.dma_start(fc2b_c[:], fc2_b[:].rearrange("(o a) -> o a", a=1))

        # edge multiplicity [dst-local (partition), graph, src-local]
        A_sb = sp.tile([128, G, L], F32, tag="acnt")
        nc.sync.dma_start(A_sb[:], a_d[:, :, :].rearrange("g p s -> p g s"))

        def load_w(dram, rows=None):
            t = wp.tile([128, KT, D], F32R, tag="w")
            src = dram[:, :] if rows is None else dram[rows[0]:rows[1], :]
            nc.sync.dma_start(t[:], src.rearrange("(t p) o -> p t o", p=128))
            return t

        # ---- input xT ----
        xT = actp.tile([128, KT, NPC], F32R, tag="act")
        nc.sync.dma_start(xT[:], xT_d[:, :].rearrange("(t p) n -> p t n", p=128))

        # =========== one TransformerConv layer ===========
        def conv_layer(lidx, actT):
            l = str(lidx)
            wq = load_w(wd["wq" + l])
            wk = load_w(wd["wk" + l])
            wv = load_w(wd["wv" + l])
            ws = load_w(wd["ws" + l])

            # --- qT, kT (transposed, bf16, q pre-scaled by 1/sqrt(C)) ---
            qT = qkp.tile([128, KT, NPC], BF16, tag="qT")
            kT = qkp.tile([128, KT, NPC], BF16, tag="kT")
            for w_sb, o_sb, scale, bias in (
                (wq, qT, SCALE, bias_c["bqs" + l]),
                (wk, kT, 1.0, bias_c["bk" + l]),
            ):
                for dt in range(KT):
                    for ch in range(NCH):
                        ps = pp.tile([128, 512], F32, tag="bank")
                        for kt in range(KT):
                            nc.tensor.matmul(
                                ps[:],
                                w_sb[:, kt, dt * 128:(dt + 1) * 128],
                                actT[:, kt, ch * 512:(ch + 1) * 512],
                                start=(kt == 0), stop=(kt == KT - 1))
                        nc.scalar.activation(
                            o_sb[:, dt, ch * 512:(ch + 1) * 512], ps[:],
                            AF.Identity, bias=bias[:, dt:dt + 1], scale=scale)

            # --- V natural [node, D] bf16 (bias via ones-row matmul) ---
            v_sb = qkp.tile([128, G, D], BF16, tag="v")
            for g in range(G):
                for chv in range(2):
                    ps = pp.tile([128, 384], F32, tag="bank")
                    for kt in range(KT):
                        nc.tensor.matmul(
                            ps[:],
                            actT[:, kt, g * 128:(g + 1) * 128],
                            wv[:, kt, chv * 384:(chv + 1) * 384],
                            start=(kt == 0), stop=False)
                    nc.tensor.matmul(
                        ps[:], ones_row[:],
                        bias_c["bv" + l][:, chv * 384:(chv + 1) * 384],
                        start=False, stop=True)
                    nc.scalar.copy(v_sb[:, g, chv * 384:(chv + 1) * 384], ps[:])

            # --- skip (x @ ws) into wide psum banks; attention adds in ---
            hT = actp.tile([128, KT, NPC], F32R, tag="act")
            for ch in range(NCH):
                banks = []
                for dt in range(KT):
                    ps = pp.tile([128, 512], F32, tag="bank")
                    for kt in range(KT):
                        nc.tensor.matmul(
                            ps[:],
                            ws[:, kt, dt * 128:(dt + 1) * 128],
                            actT[:, kt, ch * 512:(ch + 1) * 512],
                            start=(kt == 0), stop=False)
                    banks.append(ps)

                # --- attention for the 4 graphs of this chunk ---
                for gl in range(4):
                    g = ch * 4 + gl
                    for h in range(H):
                        psS = pp.tile([128, 128], F32, tag="bank")
                        for ct in range(CT):
                            dti = h * CT + ct
                            nc.tensor.matmul(
                                psS[:],
                                qT[:, dti, g * 128:(g + 1) * 128],
                                kT[:, dti, g * 128:(g + 1) * 128],
                                start=(ct == 0), stop=(ct == CT - 1))
                        negM = ap_.tile([128, 1], F32, tag="negM")
                        nc.vector.reduce_max(negM[:], psS[:], axis=AX.X,
                                             negate=True)
                        Et = ap_.tile([128, 128], F32, tag="E")
                        nc.scalar.activation(Et[:], psS[:], AF.Exp,
                                             bias=negM[:])
                        Pt = ap_.tile([128, 128], F32, tag="P")
                        nc.vector.tensor_mul(Pt[:], Et[:], A_sb[:, g, :])
                        Z = ap_.tile([128, 1], F32, tag="Z")
                        nc.vector.reduce_sum(Z[:], Pt[:], axis=AX.X)
                        nc.vector.tensor_scalar_max(Z[:], Z[:], 1e-30)
                        r = ap_.tile([128, 1], F32, tag="r")
                        nc.vector.reciprocal(r[:], Z[:])
                        Pn = ap_.tile([128, 128], BF16, tag="Pn")
                        nc.vector.tensor_scalar(Pn[:], Pt[:], r[:], None,
                                                ALU.mult)
                        psT = pp.tile([128, 128], BF16, tag="bank")
                        nc.tensor.transpose(psT[:], Pn[:], eyeB[:])
                        PT = ap_.tile([128, 128], BF16, tag="PT")
                        nc.scalar.copy(PT[:], psT[:])
                        for ct in range(CT):
                            dti = h * CT + ct
                            nc.tensor.matmul(
                                banks[dti][:, gl * 128:(gl + 1) * 128],
                                v_sb[:, g, dti * 128:(dti + 1) * 128],
                                PT[:],
                                start=False, stop=(gl == 3))

                # --- evacuate: h = relu(skip + agg + bs) ---
                for dt in range(KT):
                    nc.scalar.activation(
                        hT[:, dt, ch * 512:(ch + 1) * 512], banks[dt][:],
                        AF.Relu, bias=bias_c["bs" + l][:, dt:dt + 1])
            return hT

        h1T = conv_layer(1, xT)
        h2T = conv_layer(2, h1T)

        # =========== attention pooling + head ===========
        attiT = load_w(atti_w, rows=(0, D))       # x_q part
        attiB = load_w(atti_w, rows=(D, 2 * D))   # h part

        # xc = relu(x_q @ Wt + h @ Wb + atti_b)   (transposed, bf16)
        xcT = qkp.tile([128, KT, NPC], BF16, tag="v")  # reuse v slot
        for dt in range(KT):
            for ch in range(NCH):
                ps = pp.tile([128, 512], F32, tag="bank")
                for kt in range(KT):
                    nc.tensor.matmul(
                        ps[:],
                        attiB[:, kt, dt * 128:(dt + 1) * 128],
                        h2T[:, kt, ch * 512:(ch + 1) * 512],
                        start=(kt == 0), stop=False)
                for kt in range(KT):
                    nc.tensor.matmul(
                        ps[:],
                        attiT[:, kt, dt * 128:(dt + 1) * 128],
                        _bcast_cols(h2T, kt, ch * 512, 4, 128),
                        start=False, stop=(kt == KT - 1))
                nc.scalar.activation(
                    xcT[:, dt, ch * 512:(ch + 1) * 512], ps[:],
                    AF.Relu, bias=attib_c[:, dt:dt + 1])

        # h2 natural (bf16) via PE transposes, for pooled = h2^T p
        h2n = actp.tile([128, G, D], BF16, tag="act")
        for g in range(G):
            for dt in range(KT):
                ps = pp.tile([128, 128], F32R, tag="bank")
                nc.tensor.transpose(ps[:], h2T[:, dt, g * 128:(g + 1) * 128],
                                    eyeF[:])
                nc.scalar.copy(h2n[:, g, dt * 128:(dt + 1) * 128],
                               ps[:].bitcast(F32))

        # per-graph score softmax (column layout) -> pcols [128, G] bf16
        pcols = sp.tile([128, G], BF16, tag="pcols")
        for g in range(G):
            psSc = pp.tile([128, 1], F32, tag="bank")
            for kt in range(KT):
                nc.tensor.matmul(psSc[:],
                                 xcT[:, kt, g * 128:(g + 1) * 128],
                                 attsw_b[:, kt:kt + 1],
                                 start=(kt == 0), stop=(kt == KT - 1))
            Es = ap_.tile([128, 1], F32, tag="Es")
            nc.scalar.activation(Es[:], psSc[:], AF.Exp)
            psZ = pp.tile([1, 1], F32, tag="bank")
            nc.tensor.matmul(psZ[:], ones_col_f[:], Es[:],
                             start=True, stop=True)
            Zs = ap_.tile([1, 1], F32, tag="Zs")
            nc.scalar.copy(Zs[:], psZ[:])
            psZb = pp.tile([128, 1], F32, tag="bank")
            nc.tensor.matmul(psZb[:], ones_row_f[:], Zs[:],
                             start=True, stop=True)
            rp = ap_.tile([128, 1], F32, tag="rp")
            nc.vector.reciprocal(rp[:], psZb[:])
            nc.vector.tensor_mul(pcols[:, g:g + 1], Es[:], rp[:])

        # pooledT[dout, g] = sum_n h2[n, dout] * p[n, g]
        pooledT = sp.tile([128, KT, G], F32R, tag="pooledT")
        for dt in range(KT):
            ps = pp.tile([128, G], F32, tag="bank")
            for g in range(G):
                nc.tensor.matmul(ps[:, g:g + 1],
                                 h2n[:, g, dt * 128:(dt + 1) * 128],
                                 pcols[:, g:g + 1], start=True, stop=True)
            nc.scalar.copy(pooledT[:, dt, :], ps[:])

        # fc1 + tanh (transposed)
        fc1w = load_w(fc1_w)
        z1 = sp.tile([128, KT, G], F32R, tag="z1")
        for dt in range(KT):
            ps = pp.tile([128, G], F32, tag="bank")
            for kt in range(KT):
                nc.tensor.matmul(ps[:],
                                 fc1w[:, kt, dt * 128:(dt + 1) * 128],
                                 pooledT[:, kt, :],
                                 start=(kt == 0), stop=(kt == KT - 1))
            nc.scalar.activation(z1[:, dt, :], ps[:], AF.Tanh,
                                 bias=fc1b_c[:, dt:dt + 1])

        # fc2 -> [3, G] -> transpose -> log_softmax -> out
        psO = pp.tile([3, G], F32, tag="bank")
        for kt in range(KT):
            nc.tensor.matmul(psO[:], fc2w_c[:, kt, :], z1[:, kt, :],
                             start=(kt == 0), stop=(kt == KT - 1))
        oT = sp.tile([3, G], F32, tag="oT")
        nc.scalar.activation(oT[:], psO[:], AF.Identity, bias=fc2b_c[:])
        psOt = pp.tile([G, 3], F32, tag="bank")
        nc.tensor.transpose(psOt[:], oT[:], eyeF[0:3, 0:3].bitcast(F32))
        nm = ap_.tile([G, 1], F32, tag="nm")
        nc.vector.reduce_max(nm[:], psOt[:].bitcast(F32), axis=AX.X, negate=True)
        eo = ap_.tile([G, 3], F32, tag="eo")
        zo = ap_.tile([G, 1], F32, tag="zo")
        nc.scalar.activation(eo[:], psOt[:].bitcast(F32), AF.Exp, bias=nm[:],
                             accum_out=zo[:])
        lz = ap_.tile([G, 1], F32, tag="lz")
        nc.scalar.activation(lz[:], zo[:], AF.Ln)
        t1 = ap_.tile([G, 3], F32, tag="t1")
        nc.vector.tensor_scalar(t1[:], psOt[:].bitcast(F32), nm[:], None, ALU.add)
        ofin = ap_.tile([G, 3], F32, tag="ofin")
        nc.vector.tensor_scalar(ofin[:], t1[:], lz[:], None, ALU.subtract)
        nc.sync.dma_start(out_d[:, :], ofin[:])

    nc.compile()
    return nc


def _get_program():
    if "nc" not in _CACHE:
        _CACHE["nc"] = _build_program()
    return _CACHE["nc"]


def make_in_maps(inputs):
    x = np.asarray(inputs["x"], np.float32)
    ei = np.asarray(inputs["edge_index"])
    src, dst = ei[0].astype(np.int64), ei[1].astype(np.int64)
    # A[graph, dst_local, src_local] edge counts; edges are within-graph
    flat = dst * L + (src % L)
    acnt = np.bincount(flat, minlength=N * L).reshape(B, L, L).astype(np.float32)

    shared = {}
    for l in ("1", "2"):
        for w in ("wq", "wk", "wv", "ws"):
            shared[w + l] = np.ascontiguousarray(np.asarray(inputs[w + l], np.float32))
        for b in ("bq", "bk", "bv", "bs"):
            shared[b + l] = np.ascontiguousarray(np.asarray(inputs[b + l], np.float32))
    for nme in ("atti_w", "atti_b", "atts_w", "fc1_w", "fc1_b", "fc2_w", "fc2_b"):
        shared[nme] = np.ascontiguousarray(np.asarray(inputs[nme], np.float32))
    shared["eye_f"] = np.eye(128, dtype=np.float32)
    shared["ones_d"] = np.ones((128, 128), np.float32)
    shared["eye_b"] = np.eye(128, dtype=ml_dtypes.bfloat16)

    in_maps = []
    for c in range(N_CORES):
        m = dict(shared)
        m["xT"] = np.ascontiguousarray(x[c * NPC:(c + 1) * NPC].T)
        m["acnt"] = np.ascontiguousarray(acnt[c * G:(c + 1) * G])
        in_maps.append(m)
    return in_maps


def kernel(**inputs):
    nc = _get_program()
    in_maps = make_in_maps(inputs)
    res = run_bass_kernel_spmd(nc, in_maps, core_ids=list(range(N_CORES)))
    out = np.concatenate([res.results[c]["out"] for c in range(N_CORES)], axis=0)
    return out.astype(np.float32)


# revision 11
# speedup vs baseline: 60.8691x; 60.8691x over previous
"""Trainium2 Bass kernel for a 2-layer TransformerConv GNN + attention pooling.

Strategy: 64 equal graphs of 128 nodes; edges are within-graph. Shard 8
graphs per NeuronCore (batch sharding). Per graph, the scatter-softmax
attention over edges is computed DENSELY as masked attention with an edge
multiplicity matrix A[dst, src] (edge counts), so everything is PE matmuls:

  S = (Q/sqrt(C)) K^T  (per head)           -> PE
  P = A * exp(S - rowmax(S)); Z = rowsum(P) -> ACT/DVE
  agg = (P/Z) @ V                           -> PE (via P^T transpose)

Activations are kept TRANSPOSED [D, nodes] so natural-layout weights are
the stationary matmul operand. float32r matmuls run at 1 cycle/row for
free-dim >= 256 with near-fp32 precision; attention internals use bf16.
"""

import sys
import numpy as np

if "/opt/trn_rl_repo" not in sys.path:
    sys.path.insert(0, "/opt/trn_rl_repo")

import ml_dtypes  # noqa: F401
import concourse.bacc as bacc
import concourse.bass as bass
import concourse.mybir as mybir
import concourse.tile as tile
from concourse.bass_utils import run_bass_kernel_spmd

F32 = mybir.dt.float32
F32R = mybir.dt.float32r
BF16 = mybir.dt.bfloat16
AF = mybir.ActivationFunctionType
AX = mybir.AxisListType
ALU = mybir.AluOpType

# problem constants (hardcoded per contract)
B, L, D, H, E = 64, 128, 768, 2, 131072
N = B * L                 # 8192 nodes
C = D // H                # 384 per-head channels
N_CORES = 8
G = B // N_CORES          # 8 graphs per core
NPC = G * L               # 1024 nodes per core
KT = D // 128             # 6 k-tiles of 128
CT = C // 128             # 3 c-tiles per head
NCH = NPC // 512          # 2 node chunks of 512
SCALE = 1.0 / float(np.sqrt(C))

_CACHE = {}


def _bcast_cols(t, kt, col0, ngraph, rep):
    """AP over tile [128, KT, NPC] reading column col0 + g*L for each of
    `ngraph` graphs, each repeated `rep` times (step-0 inner dim)."""
    full = t[:]
    pstep = full.ap[0][0]
    off = full.offset + kt * NPC + col0
    return bass.AP(full.tensor, off, [[pstep, 128], [L, ngraph], [0, rep]])


def _build_program(repeat=1):
    nc = bacc.Bacc("TRN2", target_bir_lowering=False)

    # ---- DRAM I/O ----
    xT_d = nc.dram_tensor("xT", [D, NPC], F32R, kind="ExternalInput")
    a_d = nc.dram_tensor("acnt", [G, L, L], F32, kind="ExternalInput")
    wd = {}
    for l in ("1", "2"):
        for w in ("wq", "wk", "wv", "ws"):
            wd[w + l] = nc.dram_tensor(w + l, [D, D], F32R, kind="ExternalInput")
        for b in ("bq", "bk", "bs"):
            wd[b + l] = nc.dram_tensor(b + l, [D], F32, kind="ExternalInput")
        wd["bv" + l] = nc.dram_tensor("bv" + l, [D], F32R, kind="ExternalInput")
    atti_w = nc.dram_tensor("atti_w", [2 * D, D], F32R, kind="ExternalInput")
    atti_b = nc.dram_tensor("atti_b", [D], F32, kind="ExternalInput")
    atts_w = nc.dram_tensor("atts_w", [D, 1], F32, kind="ExternalInput")
    fc1_w = nc.dram_tensor("fc1_w", [D, D], F32R, kind="ExternalInput")
    fc1_b = nc.dram_tensor("fc1_b", [D], F32, kind="ExternalInput")
    fc2_w = nc.dram_tensor("fc2_w", [D, 3], F32R, kind="ExternalInput")
    fc2_b = nc.dram_tensor("fc2_b", [3], F32, kind="ExternalInput")
    eye_f = nc.dram_tensor("eye_f", [128, 128], F32R, kind="ExternalInput")
    ones_d = nc.dram_tensor("ones_d", [128, 128], F32R, kind="ExternalInput")
    eye_b = nc.dram_tensor("eye_b", [128, 128], BF16, kind="ExternalInput")
    tick_d = nc.dram_tensor("tick", [G, 3], F32, kind="ExternalInput")
    out_d = nc.dram_tensor("out", [G, 3], F32, kind="ExternalOutput")

    with tile.TileContext(nc) as tc, (
        tc.tile_pool(name="wpool", bufs=3)) as wp, (
        tc.tile_pool(name="act", bufs=3)) as actp, (
        tc.tile_pool(name="qk", bufs=1)) as qkp, (
        tc.tile_pool(name="small", bufs=1)) as sp, (
        tc.tile_pool(name="attn", bufs=4)) as ap_, (
        tc.tile_pool(name="psum", bufs=8, space=bass.MemorySpace.PSUM)) as pp:

        # ---- constants / small tiles (loaded once) ----
        tick_t = sp.tile([G, 3], F32, tag="tick")
        nc.sync.dma_start(tick_t[:], tick_d[:, :])
        eyeF = sp.tile([128, 128], F32R, tag="eyeF")
        nc.sync.dma_start(eyeF[:], eye_f[:, :])
        eyeB = sp.tile([128, 128], BF16, tag="eyeB")
        nc.sync.dma_start(eyeB[:], eye_b[:, :])
        ones_row = sp.tile([1, 128], F32R, tag="ones_row")
        nc.sync.dma_start(ones_row[:], ones_d[0:1, :])
        ones_row_f = sp.tile([1, 128], F32, tag="ones_row_f")
        nc.sync.dma_start(ones_row_f[:], ones_d[0:1, :].bitcast(F32))
        ones_col_f = sp.tile([128, 1], F32, tag="ones_col_f")
        nc.sync.dma_start(ones_col_f[:], ones_d[:, 0:1].bitcast(F32))

        def load_bias_cols(name, dram):
            t = sp.tile([128, KT], F32, tag="b_" + name)
            nc.sync.dma_start(t[:], dram[:].rearrange("(t p) -> p t", p=128))
            return t

        bias_c = {}
        for l in ("1", "2"):
            for b in ("bq", "bk", "bs"):
                bias_c[b + l] = load_bias_cols(b + l, wd[b + l])
            t = sp.tile([1, D], F32R, tag="br_bv" + l)
            nc.sync.dma_start(t[:], wd["bv" + l][:].rearrange("(a d) -> a d", a=1))
            bias_c["bv" + l] = t
            # pre-scaled bq for folding 1/sqrt(C) into q
            t = sp.tile([128, KT], F32, tag="bqs" + l)
            nc.vector.tensor_scalar_mul(t[:], bias_c["bq" + l][:], SCALE)
            bias_c["bqs" + l] = t
        attib_c = load_bias_cols("attib", atti_b)
        attsw_c = sp.tile([128, KT], F32, tag="attsw")
        nc.sync.dma_start(attsw_c[:],
                          atts_w[:, :].rearrange("(t p) o -> p (t o)", p=128))
        attsw_b = sp.tile([128, KT], BF16, tag="attswb")
        nc.vector.tensor_copy(attsw_b[:], attsw_c[:])
        fc1b_c = load_bias_cols("fc1b", fc1_b)
        fc2w_c = sp.tile([128, KT, 3], F32R, tag="fc2w")
        nc.sync.dma_start(fc2w_c[:],
                          fc2_w[:, :].rearrange("(t p) o -> p t o", p=128))
        fc2b_c = sp.tile([3, 1], F32, tag="fc2b")
        nc.sync.dma_start(fc2b_c[:], fc2_b[:].rearrange("(o a) -> o a", a=1))

        # edge multiplicity [dst-local (partition), graph, src-local]
        A_sb = sp.tile([128, G, L], F32, tag="acnt")
        nc.sync.dma_start(A_sb[:], a_d[:, :, :].rearrange("g p s -> p g s"))

        def load_w(dram, rows=None):
            t = wp.tile([128, KT, D], F32R, tag="w")
            r0 = 0 if rows is None else rows[0]
            for kt in range(KT):
                nc.sync.dma_start(t[:, kt, :],
                                  dram[r0 + kt * 128:r0 + (kt + 1) * 128, :])
            return t

        def forward():
            # ---- input xT (split DMAs so queues parallelize) ----
            xT = actp.tile([128, KT, NPC], F32R, tag="act")
            for kt in range(KT):
                nc.sync.dma_start(xT[:, kt, :],
                                  xT_d[kt * 128:(kt + 1) * 128, :])

            # =========== one TransformerConv layer ===========
            def conv_layer(lidx, actT):
                l = str(lidx)
                wq = load_w(wd["wq" + l])
                wk = load_w(wd["wk" + l])
                wv = load_w(wd["wv" + l])
                ws = load_w(wd["ws" + l])

                # --- qT, kT (transposed, bf16, q pre-scaled by 1/sqrt(C)) ---
                qT = qkp.tile([128, KT, NPC], BF16, tag="qT")
                kT = qkp.tile([128, KT, NPC], BF16, tag="kT")
                for w_sb, o_sb, scale, bias in (
                    (wq, qT, SCALE, bias_c["bqs" + l]),
                    (wk, kT, 1.0, bias_c["bk" + l]),
                ):
                    for dt in range(KT):
                        for ch in range(NCH):
                            ps = pp.tile([128, 512], F32, tag="bank")
                            for kt in range(KT):
                                nc.tensor.matmul(
                                    ps[:],
                                    w_sb[:, kt, dt * 128:(dt + 1) * 128],
                                    actT[:, kt, ch * 512:(ch + 1) * 512],
                                    start=(kt == 0), stop=(kt == KT - 1))
                            nc.scalar.activation(
                                o_sb[:, dt, ch * 512:(ch + 1) * 512], ps[:],
                                AF.Identity, bias=bias[:, dt:dt + 1], scale=scale)

                # --- V natural [node, D] bf16 (bias via ones-row matmul) ---
                v_sb = qkp.tile([128, G, D], BF16, tag="v")
                for g in range(G):
                    for chv in range(2):
                        ps = pp.tile([128, 384], F32, tag="bank")
                        for kt in range(KT):
                            nc.tensor.matmul(
                                ps[:],
                                actT[:, kt, g * 128:(g + 1) * 128],
                                wv[:, kt, chv * 384:(chv + 1) * 384],
                                start=(kt == 0), stop=False)
                        nc.tensor.matmul(
                            ps[:], ones_row[:],
                            bias_c["bv" + l][:, chv * 384:(chv + 1) * 384],
                            start=False, stop=True)
                        nc.vector.tensor_copy(
                            v_sb[:, g, chv * 384:(chv + 1) * 384], ps[:])

                # --- skip (x @ ws) into wide psum banks; attention adds in ---
                hT = actp.tile([128, KT, NPC], F32R, tag="act")
                for ch in range(NCH):
                    banks = []
                    for dt in range(KT):
                        ps = pp.tile([128, 512], F32, tag="bank")
                        for kt in range(KT):
                            nc.tensor.matmul(
                                ps[:],
                                ws[:, kt, dt * 128:(dt + 1) * 128],
                                actT[:, kt, ch * 512:(ch + 1) * 512],
                                start=(kt == 0), stop=False)
                        banks.append(ps)

                    # --- attention for the 4 graphs of this chunk ---
                    for gl in range(4):
                        g = ch * 4 + gl
                        for h in range(H):
                            psS = pp.tile([128, 128], F32, tag="bank")
                            for ct in range(CT):
                                dti = h * CT + ct
                                nc.tensor.matmul(
                                    psS[:],
                                    qT[:, dti, g * 128:(g + 1) * 128],
                                    kT[:, dti, g * 128:(g + 1) * 128],
                                    start=(ct == 0), stop=(ct == CT - 1))
                            negM = ap_.tile([128, 1], F32, tag="negM")
                            nc.vector.reduce_max(negM[:], psS[:], axis=AX.X,
                                                 negate=True)
                            Et = ap_.tile([128, 128], F32, tag="E")
                            nc.scalar.activation(Et[:], psS[:], AF.Exp,
                                                 bias=negM[:])
                            Pt = ap_.tile([128, 128], F32, tag="P")
                            nc.vector.tensor_mul(Pt[:], Et[:], A_sb[:, g, :])
                            Z = ap_.tile([128, 1], F32, tag="Z")
                            nc.vector.reduce_sum(Z[:], Pt[:], axis=AX.X)
                            nc.vector.tensor_scalar_max(Z[:], Z[:], 1e-30)
                            r = ap_.tile([128, 1], F32, tag="r")
                            nc.vector.reciprocal(r[:], Z[:])
                            Pn = ap_.tile([128, 128], BF16, tag="Pn")
                            nc.vector.tensor_scalar(Pn[:], Pt[:], r[:], None,
                                                    ALU.mult)
                            psT = pp.tile([128, 128], BF16, tag="bank")
                            nc.tensor.transpose(psT[:], Pn[:], eyeB[:])
                            PT = ap_.tile([128, 128], BF16, tag="PT")
                            nc.scalar.copy(PT[:], psT[:])
                            for ct in range(CT):
                                dti = h * CT + ct
                                nc.tensor.matmul(
                                    banks[dti][:, gl * 128:(gl + 1) * 128],
                                    v_sb[:, g, dti * 128:(dti + 1) * 128],
                                    PT[:],
                                    start=False, stop=(gl == 3))

                    # --- evacuate: h = relu(skip + agg + bs) ---
                    for dt in range(KT):
                        nc.scalar.activation(
                            hT[:, dt, ch * 512:(ch + 1) * 512], banks[dt][:],
                            AF.Relu, bias=bias_c["bs" + l][:, dt:dt + 1])
                return hT

            h1T = conv_layer(1, xT)
            h2T = conv_layer(2, h1T)

            # =========== attention pooling + head ===========
            attiT = load_w(atti_w, rows=(0, D))       # x_q part
            attiB = load_w(atti_w, rows=(D, 2 * D))   # h part

            # xc = relu(x_q @ Wt + h @ Wb + atti_b)   (transposed, bf16)
            xcT = qkp.tile([128, KT, NPC], BF16, tag="v")  # reuse v slot
            for dt in range(KT):
                for ch in range(NCH):
                    ps = pp.tile([128, 512], F32, tag="bank")
                    for kt in range(KT):
                        nc.tensor.matmul(
                            ps[:],
                            attiB[:, kt, dt * 128:(dt + 1) * 128],
                            h2T[:, kt, ch * 512:(ch + 1) * 512],
                            start=(kt == 0), stop=False)
                    for kt in range(KT):
                        nc.tensor.matmul(
                            ps[:],
                            attiT[:, kt, dt * 128:(dt + 1) * 128],
                            _bcast_cols(h2T, kt, ch * 512, 4, 128),
                            start=False, stop=(kt == KT - 1))
                    nc.scalar.activation(
                        xcT[:, dt, ch * 512:(ch + 1) * 512], ps[:],
                        AF.Relu, bias=attib_c[:, dt:dt + 1])

            # h2 natural (bf16) via PE transposes, for pooled = h2^T p
            h2n = actp.tile([128, G, D], BF16, tag="act")
            for g in range(G):
                for dt in range(KT):
                    ps = pp.tile([128, 128], F32R, tag="bank")
                    nc.tensor.transpose(ps[:], h2T[:, dt, g * 128:(g + 1) * 128],
                                        eyeF[:])
                    nc.vector.tensor_copy(h2n[:, g, dt * 128:(dt + 1) * 128],
                                          ps[:].bitcast(F32))

            # per-graph score softmax (column layout) -> pcols [128, G] bf16
            pcols = sp.tile([128, G], BF16, tag="pcols")
            for g in range(G):
                psSc = pp.tile([128, 1], F32, tag="bank")
                for kt in range(KT):
                    nc.tensor.matmul(psSc[:],
                                     xcT[:, kt, g * 128:(g + 1) * 128],
                                     attsw_b[:, kt:kt + 1],
                                     start=(kt == 0), stop=(kt == KT - 1))
                Es = ap_.tile([128, 1], F32, tag="Es")
                nc.scalar.activation(Es[:], psSc[:], AF.Exp)
                psZ = pp.tile([1, 1], F32, tag="bank")
                nc.tensor.matmul(psZ[:], ones_col_f[:], Es[:],
                                 start=True, stop=True)
                Zs = ap_.tile([1, 1], F32, tag="Zs")
                nc.scalar.copy(Zs[:], psZ[:])
                psZb = pp.tile([128, 1], F32, tag="bank")
                nc.tensor.matmul(psZb[:], ones_row_f[:], Zs[:],
                                 start=True, stop=True)
                rp = ap_.tile([128, 1], F32, tag="rp")
                nc.vector.reciprocal(rp[:], psZb[:])
                nc.vector.tensor_mul(pcols[:, g:g + 1], Es[:], rp[:])

            # pooledT[dout, g] = sum_n h2[n, dout] * p[n, g]
            pooledT = sp.tile([128, KT, G], F32R, tag="pooledT")
            for dt in range(KT):
                ps = pp.tile([128, G], F32, tag="bank")
                for g in range(G):
                    nc.tensor.matmul(ps[:, g:g + 1],
                                     h2n[:, g, dt * 128:(dt + 1) * 128],
                                     pcols[:, g:g + 1], start=True, stop=True)
                nc.scalar.copy(pooledT[:, dt, :], ps[:])

            # fc1 + tanh (transposed)
            fc1w = load_w(fc1_w)
            z1 = sp.tile([128, KT, G], F32R, tag="z1")
            for dt in range(KT):
                ps = pp.tile([128, G], F32, tag="bank")
                for kt in range(KT):
                    nc.tensor.matmul(ps[:],
                                     fc1w[:, kt, dt * 128:(dt + 1) * 128],
                                     pooledT[:, kt, :],
                                     start=(kt == 0), stop=(kt == KT - 1))
                nc.scalar.activation(z1[:, dt, :], ps[:], AF.Tanh,
                                     bias=fc1b_c[:, dt:dt + 1])

            # fc2 -> [3, G] -> transpose -> log_softmax -> out
            psO = pp.tile([3, G], F32, tag="bank")
            for kt in range(KT):
                nc.tensor.matmul(psO[:], fc2w_c[:, kt, :], z1[:, kt, :],
                                 start=(kt == 0), stop=(kt == KT - 1))
            oT = sp.tile([3, G], F32, tag="oT")
            nc.scalar.activation(oT[:], psO[:], AF.Identity, bias=fc2b_c[:])
            psOt = pp.tile([G, 3], F32, tag="bank")
            nc.tensor.transpose(psOt[:], oT[:], eyeF[0:3, 0:3].bitcast(F32))
            nm = ap_.tile([G, 1], F32, tag="nm")
            nc.vector.reduce_max(nm[:], psOt[:], axis=AX.X, negate=True)
            eo = ap_.tile([G, 3], F32, tag="eo")
            zo = ap_.tile([G, 1], F32, tag="zo")
            nc.scalar.activation(eo[:], psOt[:], AF.Exp, bias=nm[:],
                                 accum_out=zo[:])
            lz = ap_.tile([G, 1], F32, tag="lz")
            nc.scalar.activation(lz[:], zo[:], AF.Ln)
            t1 = ap_.tile([G, 3], F32, tag="t1")
            nc.vector.tensor_scalar(t1[:], psOt[:], nm[:], None, ALU.add)
            ofin = ap_.tile([G, 3], F32, tag="ofin")
            nc.vector.tensor_scalar(ofin[:], t1[:], lz[:], None, ALU.subtract)
            nc.sync.dma_start(out_d[:, :], ofin[:])

        for _ in range(repeat):
            forward()

    nc.compile()
    return nc


def _get_program(repeat=1):
    key = ("nc", repeat)
    if key not in _CACHE:
        _CACHE[key] = _build_program(repeat)
    return _CACHE[key]


def make_in_maps(inputs):
    x = np.asarray(inputs["x"], np.float32)
    ei = np.asarray(inputs["edge_index"])
    src, dst = ei[0].astype(np.int64), ei[1].astype(np.int64)
    # A[graph, dst_local, src_local] edge counts; edges are within-graph
    flat = dst * L + (src % L)
    acnt = np.bincount(flat, minlength=N * L).reshape(B, L, L).astype(np.float32)

    shared = {}
    for l in ("1", "2"):
        for w in ("wq", "wk", "wv", "ws"):
            shared[w + l] = np.ascontiguousarray(np.asarray(inputs[w + l], np.float32))
        for b in ("bq", "bk", "bv", "bs"):
            shared[b + l] = np.ascontiguousarray(np.asarray(inputs[b + l], np.float32))
    for nme in ("atti_w", "atti_b", "atts_w", "fc1_w", "fc1_b", "fc2_w", "fc2_b"):
        shared[nme] = np.ascontiguousarray(np.asarray(inputs[nme], np.float32))
    shared["eye_f"] = np.eye(128, dtype=np.float32)
    shared["ones_d"] = np.ones((128, 128), np.float32)
    shared["eye_b"] = np.eye(128, dtype=ml_dtypes.bfloat16)

    in_maps = []
    for c in range(N_CORES):
        m = dict(shared)
        m["tick"] = np.zeros((G, 3), np.float32)
        m["xT"] = np.ascontiguousarray(x[c * NPC:(c + 1) * NPC].T)
        m["acnt"] = np.ascontiguousarray(acnt[c * G:(c + 1) * G])
        in_maps.append(m)
    return in_maps


def kernel(**inputs):
    nc = _get_program()
    in_maps = make_in_maps(inputs)
    res = run_bass_kernel_spmd(nc, in_maps, core_ids=list(range(N_CORES)))
    out = np.concatenate([res.results[c]["out"] for c in range(N_CORES)], axis=0)
    return out.astype(np.float32)


# revision 24
# speedup vs baseline: 235.6603x; 3.8716x over previous
"""Trainium2 Bass kernel for a 2-layer TransformerConv GNN + attention pooling.

Strategy: 64 equal graphs of 128 nodes; edges are within-graph. Shard 8
graphs per NeuronCore (batch sharding). Per graph, the scatter-softmax
attention over edges is computed DENSELY as masked attention with an edge
multiplicity matrix A[dst, src] (edge counts), so everything is PE matmuls:

  S = (Q/sqrt(C)) K^T  (per head)           -> PE
  P = A * exp(S - rowmax(S)); Z = rowsum(P) -> ACT/DVE
  agg = (P/Z) @ V                           -> PE (via P^T transpose)

Activations are kept TRANSPOSED [D, nodes] so natural-layout weights are
the stationary matmul operand. float32r matmuls run at 1 cycle/row for
free-dim >= 256 with near-fp32 precision; attention internals use bf16.
"""

import sys
import numpy as np

if "/opt/trn_rl_repo" not in sys.path:
    sys.path.insert(0, "/opt/trn_rl_repo")

import ml_dtypes  # noqa: F401
import concourse.bacc as bacc
import concourse.bass as bass
import concourse.mybir as mybir
import concourse.tile as tile
from concourse.bass_utils import run_bass_kernel_spmd

F32 = mybir.dt.float32
F32R = mybir.dt.float32r
BF16 = mybir.dt.bfloat16
AF = mybir.ActivationFunctionType
AX = mybir.AxisListType
ALU = mybir.AluOpType

# problem constants (hardcoded per contract)
B, L, D, H, E = 64, 128, 768, 2, 131072
N = B * L                 # 8192 nodes
C = D // H                # 384 per-head channels
N_CORES = 8
G = B // N_CORES          # 8 graphs per core
NPC = G * L               # 1024 nodes per core
KT = D // 128             # 6 k-tiles of 128
CT = C // 128             # 3 c-tiles per head
NCH = NPC // 512          # 2 node chunks of 512
SCALE = 1.0 / float(np.sqrt(C))

_CACHE = {}


def _bcast_cols(t, kt, col0, ngraph, rep):
    """AP over tile [128, KT, NPC] reading column col0 + g*L for each of
    `ngraph` graphs, each repeated `rep` times (step-0 inner dim)."""
    full = t[:]
    pstep = full.ap[0][0]
    off = full.offset + kt * NPC + col0
    return bass.AP(full.tensor, off, [[pstep, 128], [L, ngraph], [0, rep]])


def _build_program(repeat=1):
    nc = bacc.Bacc("TRN2", target_bir_lowering=False)

    # ---- DRAM I/O ----
    xT_d = nc.dram_tensor("xT", [D, NPC], F32R, kind="ExternalInput")
    a_d = nc.dram_tensor("acnt", [G, L, L], F32, kind="ExternalInput")
    wd = {}
    for l in ("1", "2"):
        for w in ("wq", "wk", "wv", "ws"):
            wd[w + l] = nc.dram_tensor(w + l, [D, D], F32R, kind="ExternalInput")
        for b in ("bq", "bk", "bs"):
            wd[b + l] = nc.dram_tensor(b + l, [D], F32, kind="ExternalInput")
        wd["bv" + l] = nc.dram_tensor("bv" + l, [D], F32R, kind="ExternalInput")
    atti_w = nc.dram_tensor("atti_w", [2 * D, D], F32R, kind="ExternalInput")
    atti_b = nc.dram_tensor("atti_b", [D], F32, kind="ExternalInput")
    atts_w = nc.dram_tensor("atts_w", [D, 1], F32, kind="ExternalInput")
    fc1_w = nc.dram_tensor("fc1_w", [D, D], F32R, kind="ExternalInput")
    fc1_b = nc.dram_tensor("fc1_b", [D], F32, kind="ExternalInput")
    fc2_w = nc.dram_tensor("fc2_w", [D, 3], F32R, kind="ExternalInput")
    fc2_b = nc.dram_tensor("fc2_b", [3], F32, kind="ExternalInput")
    eye_f = nc.dram_tensor("eye_f", [128, 128], F32R, kind="ExternalInput")
    ones_d = nc.dram_tensor("ones_d", [128, 128], F32R, kind="ExternalInput")
    eye_b = nc.dram_tensor("eye_b", [128, 128], BF16, kind="ExternalInput")
    tick_d = nc.dram_tensor("tick", [G, 3], F32, kind="ExternalInput")
    out_d = nc.dram_tensor("out", [G, 3], F32, kind="ExternalOutput")

    with tile.TileContext(nc) as tc, (
        tc.tile_pool(name="wpool", bufs=5)) as wp, (
        tc.tile_pool(name="act", bufs=4)) as actp, (
        tc.tile_pool(name="qk", bufs=1)) as qkp, (
        tc.tile_pool(name="small", bufs=1)) as sp, (
        tc.tile_pool(name="attn", bufs=4)) as ap_, (
        tc.tile_pool(name="psum", bufs=8, space=bass.MemorySpace.PSUM)) as pp:

        # ---- constants / small tiles (loaded once) ----
        tick_t = sp.tile([G, 3], F32, tag="tick")
        nc.scalar.dma_start(tick_t[:], tick_d[:, :])
        eyeF = sp.tile([128, 128], F32R, tag="eyeF")
        nc.scalar.dma_start(eyeF[:], eye_f[:, :])
        eyeB = sp.tile([128, 128], BF16, tag="eyeB")
        nc.scalar.dma_start(eyeB[:], eye_b[:, :])
        ones_row = sp.tile([1, 128], F32R, tag="ones_row")
        nc.scalar.dma_start(ones_row[:], ones_d[0:1, :])
        ones_row_f = sp.tile([1, 128], F32, tag="ones_row_f")
        nc.scalar.dma_start(ones_row_f[:], ones_d[0:1, :].bitcast(F32))
        ones_col_f = sp.tile([128, 1], F32, tag="ones_col_f")
        nc.scalar.dma_start(ones_col_f[:], ones_d[:, 0:1].bitcast(F32))

        def load_bias_cols(name, dram):
            t = sp.tile([128, KT], F32, tag="b_" + name)
            nc.scalar.dma_start(t[:], dram[:].rearrange("(t p) -> p t", p=128))
            return t

        bias_c = {}
        for l in ("1", "2"):
            for b in ("bq", "bk", "bs"):
                bias_c[b + l] = load_bias_cols(b + l, wd[b + l])
            t = sp.tile([1, D], F32R, tag="br_bv" + l)
            nc.scalar.dma_start(t[:], wd["bv" + l][:].rearrange("(a d) -> a d", a=1))
            bias_c["bv" + l] = t
            # pre-scaled bq for folding 1/sqrt(C) into q
            t = sp.tile([128, KT], F32, tag="bqs" + l)
            nc.vector.tensor_scalar_mul(t[:], bias_c["bq" + l][:], SCALE)
            bias_c["bqs" + l] = t
        attib_c = load_bias_cols("attib", atti_b)
        attsw_c = sp.tile([128, KT], F32, tag="attsw")
        nc.scalar.dma_start(attsw_c[:],
                          atts_w[:, :].rearrange("(t p) o -> p (t o)", p=128))
        attsw_b = sp.tile([128, KT], BF16, tag="attswb")
        nc.vector.tensor_copy(attsw_b[:], attsw_c[:])
        fc1b_c = load_bias_cols("fc1b", fc1_b)
        fc2w_c = sp.tile([128, KT, 3], F32R, tag="fc2w")
        nc.scalar.dma_start(fc2w_c[:],
                          fc2_w[:, :].rearrange("(t p) o -> p t o", p=128))
        fc2b_c = sp.tile([3, 1], F32, tag="fc2b")
        nc.scalar.dma_start(fc2b_c[:], fc2_b[:].rearrange("(o a) -> o a", a=1))

        def load_w(dram, rows=None):
            t = wp.tile([128, KT, D], F32R, tag="w")
            r0 = 0 if rows is None else rows[0]
            for kt in range(KT):
                nc.sync.dma_start(t[:, kt, :],
                                  dram[r0 + kt * 128:r0 + (kt + 1) * 128, :])
            return t

        def forward():
            # ---- interleave layer-1 wq with xT chunks: fast first-flight ----
            xT = [actp.tile([128, KT, 512], F32R, tag="act", name=f"xT{c}")
                  for c in range(NCH)]
            wq1 = wp.tile([128, KT, D], F32R, tag="w")
            for kt in range(KT):
                nc.sync.dma_start(wq1[:, kt, :],
                                  wd["wq1"][kt * 128:(kt + 1) * 128, :])
                for ch in range(NCH):
                    nc.sync.dma_start(
                        xT[ch][:, kt, :],
                        xT_d[kt * 128:(kt + 1) * 128, ch * 512:(ch + 1) * 512])
            # ln(edge count) [dst-local (partition), graph, src-local]
            A_sb = sp.tile([128, G, L], F32, tag="acnt")
            nc.scalar.dma_start(A_sb[:], a_d[:, :, :].rearrange("g p s -> p g s"))

            # =========== one TransformerConv layer ===========
            def conv_layer(lidx, actT, wq=None):
                l = str(lidx)
                if wq is None:
                    wq = load_w(wd["wq" + l])
                wk = load_w(wd["wk" + l])
                wv = load_w(wd["wv" + l])
                ws = load_w(wd["ws" + l])

                # --- qT, kT (transposed, bf16, q pre-scaled by 1/sqrt(C)) ---
                qT = qkp.tile([128, KT, NPC], BF16, tag="qT")
                kT = qkp.tile([128, KT, NPC], BF16, tag="kT")
                for w_sb, o_sb, scale, bias in (
                    (wq, qT, SCALE, bias_c["bqs" + l]),
                    (wk, kT, 1.0, bias_c["bk" + l]),
                ):
                    for dt in range(KT):
                        for ch in range(NCH):
                            ps = pp.tile([128, 512], F32, tag="bank")
                            for kt in range(KT):
                                nc.tensor.matmul(
                                    ps[:],
                                    w_sb[:, kt, dt * 128:(dt + 1) * 128],
                                    actT[ch][:, kt, :],
                                    start=(kt == 0), stop=(kt == KT - 1))
                            nc.vector.tensor_scalar(
                                o_sb[:, dt, ch * 512:(ch + 1) * 512], ps[:],
                                bias[:, dt:dt + 1], scale, ALU.add, ALU.mult)

                # --- V natural [node, D] bf16 (bias via ones-row matmul) ---
                v_sb = qkp.tile([128, G, D], BF16, tag="v")
                for g in range(G):
                    for chv in range(2):
                        ps = pp.tile([128, 384], F32, tag="bank")
                        for kt in range(KT):
                            nc.tensor.matmul(
                                ps[:],
                                actT[g // 4][:, kt,
                                             (g % 4) * 128:(g % 4 + 1) * 128],
                                wv[:, kt, chv * 384:(chv + 1) * 384],
                                start=(kt == 0), stop=False)
                        nc.tensor.matmul(
                            ps[:], ones_row[:],
                            bias_c["bv" + l][:, chv * 384:(chv + 1) * 384],
                            start=False, stop=True)
                        nc.vector.tensor_copy(
                            v_sb[:, g, chv * 384:(chv + 1) * 384], ps[:])

                # --- skip (x @ ws) into wide psum banks; attention adds in.
                # head h touches only dt in [h*CT, (h+1)*CT) -> park 3 banks
                # at a time and run that head's attention, keeping PSUM slack.
                hT = [actp.tile([128, KT, 512], F32R, tag="act",
                                name=f"hT{l}_{c}") for c in range(NCH)]
                for ch in range(NCH):
                    for h in range(H):
                        banks = []
                        for ct in range(CT):
                            dt = h * CT + ct
                            ps = pp.tile([128, 512], F32, tag="bank")
                            for kt in range(KT):
                                nc.tensor.matmul(
                                    ps[:],
                                    ws[:, kt, dt * 128:(dt + 1) * 128],
                                    actT[ch][:, kt, :],
                                    start=(kt == 0), stop=False)
                            banks.append(ps)

                        for gl in range(4):
                            g = ch * 4 + gl
                            psS = pp.tile([128, 128], F32, tag="bank")
                            for ct in range(CT):
                                dti = h * CT + ct
                                nc.tensor.matmul(
                                    psS[:],
                                    qT[:, dti, g * 128:(g + 1) * 128],
                                    kT[:, dti, g * 128:(g + 1) * 128],
                                    start=(ct == 0), stop=(ct == CT - 1))
                            Sl = ap_.tile([128, 128], F32, tag="E")
                            nc.vector.tensor_add(Sl[:], psS[:], A_sb[:, g, :])
                            Pt = ap_.tile([128, 128], F32, tag="P")
                            Z = ap_.tile([128, 1], F32, tag="Z")
                            nc.scalar.activation(Pt[:], Sl[:], AF.Exp,
                                                 accum_out=Z[:])
                            nc.vector.tensor_scalar_max(Z[:], Z[:], 1e-30)
                            r = ap_.tile([128, 1], F32, tag="r")
                            nc.vector.reciprocal(r[:], Z[:])
                            Pn = ap_.tile([128, 128], BF16, tag="Pn")
                            nc.vector.tensor_scalar(Pn[:], Pt[:], r[:], None,
                                                    ALU.mult)
                            psT = pp.tile([128, 128], BF16, tag="bank")
                            nc.tensor.transpose(psT[:], Pn[:], eyeB[:])
                            PT = ap_.tile([128, 128], BF16, tag="PT")
                            nc.scalar.copy(PT[:], psT[:])
                            for ct in range(CT):
                                dti = h * CT + ct
                                nc.tensor.matmul(
                                    banks[ct][:, gl * 128:(gl + 1) * 128],
                                    v_sb[:, g, dti * 128:(dti + 1) * 128],
                                    PT[:],
                                    start=False, stop=(gl == 3))

                        # --- evacuate: h = relu(skip + agg + bs) ---
                        for ct in range(CT):
                            dt = h * CT + ct
                            nc.scalar.activation(
                                hT[ch][:, dt, :], banks[ct][:],
                                AF.Relu, bias=bias_c["bs" + l][:, dt:dt + 1])
                return hT

            h1T = conv_layer(1, xT, wq=wq1)
            # prefetch pooling weights; DMA overlaps layer-2 compute
            attiT = load_w(atti_w, rows=(0, D))       # x_q part
            attiB = load_w(atti_w, rows=(D, 2 * D))   # h part
            h2T = conv_layer(2, h1T)

            # =========== attention pooling + head ===========

            # Qcols: first-node columns of h2T -> [128, KT, G]
            fc1w = load_w(fc1_w)  # prefetch
            Qcols = sp.tile([128, NCH, KT, 4], F32R, tag="Qcols")
            for ch in range(NCH):
                h2full = h2T[ch][:]
                qsrc = bass.AP(h2full.tensor, h2full.offset,
                               [[h2full.ap[0][0], 128], [512, KT], [L, 4]])
                nc.scalar.dma_start(Qcols[:, ch], qsrc)
            # xc = relu(h @ Wb + cTb[g]); cTb = x_q @ Wt + atti_b
            xcT = qkp.tile([128, KT, NPC], BF16, tag="v")  # reuse v slot
            cTb = sp.tile([128, KT, G], F32, tag="cTb")
            for ch in range(NCH):
                banks = []
                for dt in range(KT):
                    ps = pp.tile([128, 512], F32, tag="bank")
                    for kt in range(KT):
                        nc.tensor.matmul(
                            ps[:],
                            attiB[:, kt, dt * 128:(dt + 1) * 128],
                            h2T[ch][:, kt, :],
                            start=(kt == 0), stop=(kt == KT - 1))
                    banks.append(ps)
                if ch == 0:
                    for dt in range(KT):
                        psc = pp.tile([128, G], F32, tag="bank")
                        for kt in range(KT):
                            nc.tensor.matmul(
                                psc[:],
                                attiT[:, kt, dt * 128:(dt + 1) * 128],
                                Qcols[:, :, kt, :],
                                start=(kt == 0), stop=(kt == KT - 1))
                        nc.scalar.activation(cTb[:, dt, :], psc[:], AF.Identity,
                                             bias=attib_c[:, dt:dt + 1])
                for dt in range(KT):
                    for gl in range(4):
                        g = ch * 4 + gl
                        nc.vector.tensor_scalar(
                            xcT[:, dt, g * 128:(g + 1) * 128],
                            banks[dt][:, gl * 128:(gl + 1) * 128],
                            cTb[:, dt, g:g + 1], 0.0, ALU.add, ALU.max)

            # h2 natural (bf16) via PE transposes, for pooled = h2^T p
            h2n = actp.tile([128, G, D], BF16, tag="h2n", bufs=1)
            for g in range(G):
                for dt in range(KT):
                    ps = pp.tile([128, 128], F32R, tag="bank")
                    nc.tensor.transpose(
                        ps[:],
                        h2T[g // 4][:, dt, (g % 4) * 128:(g % 4 + 1) * 128],
                        eyeF[:])
                    nc.vector.tensor_copy(h2n[:, g, dt * 128:(dt + 1) * 128],
                                          ps[:].bitcast(F32))

            # batched per-graph score softmax -> pcols [128, G] bf16
            psSc = pp.tile([128, G], F32, tag="bank")
            for g in range(G):
                for kt in range(KT):
                    nc.tensor.matmul(psSc[:, g:g + 1],
                                     xcT[:, kt, g * 128:(g + 1) * 128],
                                     attsw_b[:, kt:kt + 1],
                                     start=(kt == 0), stop=(kt == KT - 1))
            Es = ap_.tile([128, G], F32, tag="Es")
            nc.scalar.activation(Es[:], psSc[:], AF.Exp)
            psZ = pp.tile([1, G], F32, tag="bank")
            nc.tensor.matmul(psZ[:], ones_col_f[:], Es[:], start=True, stop=True)
            Zs = ap_.tile([1, G], F32, tag="Zs")
            nc.scalar.copy(Zs[:], psZ[:])
            psZb = pp.tile([128, G], F32, tag="bank")
            nc.tensor.matmul(psZb[:], ones_row_f[:], Zs[:], start=True, stop=True)
            rp = ap_.tile([128, G], F32, tag="rp")
            nc.vector.reciprocal(rp[:], psZb[:])
            pcols = sp.tile([128, G], BF16, tag="pcols")
            nc.vector.tensor_mul(pcols[:], Es[:], rp[:])

            # pooledT[dout, g] = sum_n h2[n, dout] * p[n, g]
            pooledT = sp.tile([128, KT, G], F32R, tag="pooledT")
            for dt in range(KT):
                ps = pp.tile([128, G], F32, tag="bank")
                for g in range(G):
                    nc.tensor.matmul(ps[:, g:g + 1],
                                     h2n[:, g, dt * 128:(dt + 1) * 128],
                                     pcols[:, g:g + 1], start=True, stop=True)
                nc.scalar.copy(pooledT[:, dt, :], ps[:])

            # fc1 + tanh (transposed)
            z1 = sp.tile([128, KT, G], F32R, tag="z1")
            for dt in range(KT):
                ps = pp.tile([128, G], F32, tag="bank")
                for kt in range(KT):
                    nc.tensor.matmul(ps[:],
                                     fc1w[:, kt, dt * 128:(dt + 1) * 128],
                                     pooledT[:, kt, :],
                                     start=(kt == 0), stop=(kt == KT - 1))
                nc.scalar.activation(z1[:, dt, :], ps[:], AF.Tanh,
                                     bias=fc1b_c[:, dt:dt + 1])

            # fc2 -> [3, G] -> transpose -> log_softmax -> out
            psO = pp.tile([3, G], F32, tag="bank")
            for kt in range(KT):
                nc.tensor.matmul(psO[:], fc2w_c[:, kt, :], z1[:, kt, :],
                                 start=(kt == 0), stop=(kt == KT - 1))
            oT = sp.tile([3, G], F32, tag="oT")
            nc.scalar.activation(oT[:], psO[:], AF.Identity, bias=fc2b_c[:])
            psOt = pp.tile([G, 3], F32, tag="bank")
            nc.tensor.transpose(psOt[:], oT[:], eyeF[0:3, 0:3].bitcast(F32))
            nm = ap_.tile([G, 1], F32, tag="nm")
            nc.vector.reduce_max(nm[:], psOt[:], axis=AX.X, negate=True)
            eo = ap_.tile([G, 3], F32, tag="eo")
            zo = ap_.tile([G, 1], F32, tag="zo")
            nc.scalar.activation(eo[:], psOt[:], AF.Exp, bias=nm[:],
                                 accum_out=zo[:])
            lz = ap_.tile([G, 1], F32, tag="lz")
            nc.scalar.activation(lz[:], zo[:], AF.Ln)
            t1 = ap_.tile([G, 3], F32, tag="t1")
            nc.vector.tensor_scalar(t1[:], psOt[:], nm[:], None, ALU.add)
            ofin = ap_.tile([G, 3], F32, tag="ofin")
            nc.vector.tensor_scalar(ofin[:], t1[:], lz[:], None, ALU.subtract)
            nc.sync.dma_start(out_d[:, :], ofin[:])

        for _ in range(repeat):
            forward()

    nc.compile()
    return nc


def _get_program(repeat=1):
    key = ("nc", repeat)
    if key not in _CACHE:
        _CACHE[key] = _build_program(repeat)
    return _CACHE[key]


def make_in_maps(inputs):
    x = np.asarray(inputs["x"], np.float32)
    ei = np.asarray(inputs["edge_index"])
    src, dst = ei[0].astype(np.int64), ei[1].astype(np.int64)
    # A[graph, dst_local, src_local] edge counts; edges are within-graph
    flat = dst * L + (src % L)
    acnt = np.bincount(flat, minlength=N * L).reshape(B, L, L).astype(np.float32)
    with np.errstate(divide="ignore"):
        acnt = np.where(acnt > 0, np.log(acnt), np.float32(-1e30)).astype(np.float32)

    shared = {}
    for l in ("1", "2"):
        for w in ("wq", "wk", "wv", "ws"):
            shared[w + l] = np.ascontiguousarray(np.asarray(inputs[w + l], np.float32))
        for b in ("bq", "bk", "bv", "bs"):
            shared[b + l] = np.ascontiguousarray(np.asarray(inputs[b + l], np.float32))
    for nme in ("atti_w", "atti_b", "atts_w", "fc1_w", "fc1_b", "fc2_w", "fc2_b"):
        shared[nme] = np.ascontiguousarray(np.asarray(inputs[nme], np.float32))
    shared["eye_f"] = np.eye(128, dtype=np.float32)
    shared["ones_d"] = np.ones((128, 128), np.float32)
    shared["eye_b"] = np.eye(128, dtype=ml_dtypes.bfloat16)

    in_maps = []
    for c in range(N_CORES):
        m = dict(shared)
        m["tick"] = np.zeros((G, 3), np.float32)
        m["xT"] = np.ascontiguousarray(x[c * NPC:(c + 1) * NPC].T)
        m["acnt"] = np.ascontiguousarray(acnt[c * G:(c + 1) * G])
        in_maps.append(m)
    return in_maps


def kernel(**inputs):
    nc = _get_program()
    in_maps = make_in_maps(inputs)
    res = run_bass_kernel_spmd(nc, in_maps, core_ids=list(range(N_CORES)))
    out = np.concatenate([res.results[c]["out"] for c in range(N_CORES)], axis=0)
    return out.astype(np.float32)
